# revision 8
# baseline (speedup 1.0000x reference)
"""Trainium2 Bass kernel for nn_ContrastiveCenterLoss_M.

Math reduction
--------------
reference computes, per sample b and class c, a Mahalanobis distance between
the pooled-normalized hidden vector x_b (8-dim) and pooled-normalized class
center y_c (8-dim), where the 8x8 covariance is over the 200 points
{x_b (repeated 100x), y_0..y_99}:

    cov_b = A + beta d_b d_b^T,  A = S_y/199,  d_b = x_b - ybar,  beta = 50/199

A depends only on feature_center and is well-conditioned (cond ~1.9), so
pinv == inv and Sherman-Morrison collapses the per-sample pinv to a rank-1
correction of the shared M = inv(A).  Working in u = x - ybar coordinates
(all class-only terms folded into host constants):

    ur_c  = u.(M y_c)                     [one 9x128^T @ 9x201 matmul]
    e0    = u.(M ybar)                    [extra matmul column]
    uw    = u.M.u = x.w - e0,  w = M u    [small matmul + elementwise]
    gamma = 1/(1/beta + uw),   sg = sqrt(gamma)
    m[b,c] = (uw + 2 e0) + (k2_c - 2 ur_c) - (sg*ur_c - sg*(uw+e0))^2
    k2_c  = (y_c-ybar).M.(y_c-ybar)       [host]
    dis = sqrt(m);  loss_b = (C*dis[b,y_b] - sum_c dis[b,c])/(C-1)

Host precomputes the tiny center-only constants in float64; the device does
all per-sample work.  Data-parallel over batch: 8 cores x 128 samples.
ACT-table sqrt measured at ~1e-6 rel on HW, so no Newton refinement.
NOTE: InstTensorTensorReduce and [p,1]-shaped DRAM outputs crash the exec
unit on this runtime -- avoided (tt+reduce pairs; [128,100] output).
"""

import sys

if "/opt/trn_rl_repo" not in sys.path:
    sys.path.insert(0, "/opt/trn_rl_repo")

import numpy as np

B = 1024
D = 512
C = 100
POOL = 8
G = D // POOL          # 64
NCORES = 8
BS = B // NCORES       # 128 samples per core
BETA = (C / 2) / (2 * C - 1)   # 50/199
NCONST = 10 + 201      # [M(8x8)+pad | ybar9 | rhsU(9x201)] packed columns

_cache = {}


def _build():
    import concourse.mybir as mybir
    import concourse.tile as tile
    from concourse import bacc
    from concourse.masks import make_identity

    f32 = mybir.dt.float32
    ALU = mybir.AluOpType
    ACT = mybir.ActivationFunctionType
    AX = mybir.AxisListType
    HALF = D // 2

    nc = bacc.Bacc(
        "TRN2",
        target_bir_lowering=False,
        debug=False,
        enable_asserts=False,
        num_devices=NCORES,
    )

    hidden_d = nc.dram_tensor("hidden_in", [BS, D], f32, kind="ExternalInput")
    const_d = nc.dram_tensor("const_in", [POOL + 1, NCONST], f32, kind="ExternalInput")
    ylab_d = nc.dram_tensor("ylab_in", [BS, 1], f32, kind="ExternalInput")
    loss_d = nc.dram_tensor("loss_out", [BS, C], f32, kind="ExternalOutput")

    with tile.TileContext(nc) as tc:
        with (
            tc.tile_pool(name="sb", bufs=1) as sb,
            tc.tile_pool(name="ps", bufs=1, space="PSUM") as ps,
        ):
            # two-half DMA so pooling overlaps the load
            h1 = sb.tile([BS, HALF], f32)
            h2 = sb.tile([BS, HALF], f32)
            nc.sync.dma_start(h1[:, :], hidden_d[:, 0:HALF])
            nc.sync.dma_start(h2[:, :], hidden_d[:, HALF:D])
            cst = sb.tile([POOL + 1, NCONST], f32)
            nc.sync.dma_start(cst[:, :], const_d[:, :])
            ylab = sb.tile([BS, 1], f32)
            nc.sync.dma_start(ylab[:, :], ylab_d[:, :])
            msb = cst[0:POOL, 0:POOL]
            ybar9 = cst[:, 9:10]
            rhsU = cst[:, 10:10 + 201]

            # constants with no deps: identity (PE transpose) + iota (one-hot)
            ident = sb.tile([BS, BS], f32)
            make_identity(nc, ident[:, :])
            io_f = sb.tile([BS, C], f32)
            nc.gpsimd.iota(out=io_f[:, :], pattern=[[1, C]], base=0,
                           channel_multiplier=0, allow_small_or_imprecise_dtypes=True)

            # ---- pool hidden into 8 groups of 64, L2-normalize -> x ----
            s8 = sb.tile([BS, POOL], f32)
            nc.vector.tensor_reduce(
                out=s8[:, 0:POOL // 2],
                in_=h1[:, :].rearrange("p (g e) -> p g e", e=G),
                axis=AX.X, op=ALU.add,
            )
            nc.vector.tensor_reduce(
                out=s8[:, POOL // 2:POOL],
                in_=h2[:, :].rearrange("p (g e) -> p g e", e=G),
                axis=AX.X, op=ALU.add,
            )
            sq = sb.tile([BS, POOL], f32)
            ss = sb.tile([BS, 1], f32)
            nc.scalar.activation(
                out=sq[:, :], in_=s8[:, :], func=ACT.Square, scale=1.0 / G,
                accum_out=ss[:, :],
            )
            nv = sb.tile([BS, 1], f32)
            nc.scalar.sqrt(out=nv[:, :], in_=ss[:, :])
            ne = sb.tile([BS, 1], f32)
            nc.vector.tensor_scalar(out=ne[:, :], in0=nv[:, :], scalar1=1e-6,
                                    scalar2=None, op0=ALU.add)
            rn = sb.tile([BS, 1], f32)
            nc.vector.reciprocal(out=rn[:, :], in_=ne[:, :])
            xn9 = sb.tile([BS, POOL + 1], f32)   # [x | 1]
            nc.vector.tensor_scalar(
                out=xn9[:, 0:POOL], in0=s8[:, :], scalar1=1.0 / G, scalar2=rn[:, 0:1],
                op0=ALU.mult, op1=ALU.mult,
            )
            nc.vector.memset(xn9[:, POOL:POOL + 1], 1.0)

            # ---- u^T (+ones row) = transpose(x|1) - (ybar|0) ----
            xnt_ps = ps.tile([POOL + 1, BS], f32)
            nc.tensor.transpose(xnt_ps[:, :], xn9[:, :], ident[:, :])
            ut9 = sb.tile([POOL + 1, BS], f32)
            nc.vector.tensor_scalar(
                out=ut9[:, :], in0=xnt_ps[:, :], scalar1=ybar9, scalar2=None,
                op0=ALU.subtract,
            )

            # ---- w = M u ; back to [128,8] ----
            wt_ps = ps.tile([POOL, BS], f32)
            nc.tensor.matmul(wt_ps[:, :], msb, ut9[0:POOL, :])
            lhsw = sb.tile([POOL, BS], f32)
            nc.scalar.copy(out=lhsw[:, :], in_=wt_ps[:, :])
            w128_ps = ps.tile([BS, POOL], f32)
            nc.tensor.transpose(w128_ps[:, :], lhsw[:, :], ident[0:POOL, 0:POOL])

            # ---- the one big contraction ----
            # cols 0:100 = u.r_c ; 100:200 = k2_c - 2 u.r_c ; 200 = e0 = u.(M ybar)
            dis_ps = ps.tile([BS, 201], f32)
            nc.tensor.matmul(dis_ps[:, :], ut9[:, :], rhsU)

            # ---- per-sample scalars ----
            xw = sb.tile([BS, POOL], f32)
            nc.vector.tensor_tensor(out=xw[:, :], in0=xn9[:, 0:POOL],
                                    in1=w128_ps[:, :], op=ALU.mult)
            xws = sb.tile([BS, 1], f32)
            nc.vector.tensor_reduce(out=xws[:, :], in_=xw[:, :], axis=AX.X, op=ALU.add)
            e0 = sb.tile([BS, 1], f32)
            nc.vector.tensor_copy(out=e0[:, :], in_=dis_ps[:, 200:201])
            uw = sb.tile([BS, 1], f32)     # u.M.u = x.w - e0
            nc.vector.tensor_scalar(out=uw[:, :], in0=xws[:, :], scalar1=e0[:, 0:1],
                                    scalar2=None, op0=ALU.subtract)
            den = sb.tile([BS, 1], f32)    # 1/beta + uw ;  gamma = 1/den
            nc.vector.tensor_scalar(out=den[:, :], in0=uw[:, :], scalar1=1.0 / BETA,
                                    scalar2=None, op0=ALU.add)
            gam = sb.tile([BS, 1], f32)
            nc.vector.reciprocal(out=gam[:, :], in_=den[:, :])
            sg = sb.tile([BS, 1], f32)
            nc.scalar.sqrt(out=sg[:, :], in_=gam[:, :])
            s2 = sb.tile([BS, 1], f32)     # uw + 2 e0
            nc.vector.tensor_scalar(out=s2[:, :], in0=e0[:, :], scalar1=2.0,
                                    scalar2=uw[:, 0:1], op0=ALU.mult, op1=ALU.add)
            t3 = sb.tile([BS, 1], f32)     # uw + e0
            nc.vector.tensor_tensor(out=t3[:, :], in0=uw[:, :], in1=e0[:, :], op=ALU.add)
            bneg = sb.tile([BS, 1], f32)   # -sg*(uw+e0)
            nc.vector.tensor_scalar(out=bneg[:, :], in0=t3[:, :], scalar1=sg[:, 0:1],
                                    scalar2=-1.0, op0=ALU.mult, op1=ALU.mult)

            # ---- m = (k2 - 2ur + s2) - (sg*ur + bneg)^2 ; dis = sqrt(m) ----
            zsq = sb.tile([BS, C], f32)
            nc.scalar.activation(out=zsq[:, :], in_=dis_ps[:, 0:C], func=ACT.Square,
                                 scale=sg[:, 0:1], bias=bneg[:, 0:1])
            d1z = sb.tile([BS, C], f32)
            nc.vector.tensor_tensor(out=d1z[:, :], in0=dis_ps[:, C:2 * C],
                                    in1=zsq[:, :], op=ALU.subtract)
            dis = sb.tile([BS, C], f32)
            rowsum = sb.tile([BS, 1], f32)
            nc.scalar.activation(out=dis[:, :], in_=d1z[:, :], func=ACT.Sqrt,
                                 bias=s2[:, 0:1], accum_out=rowsum[:, :])

            # ---- loss_b = (C/(C-1)) dis[b,y_b] - rowsum/(C-1) ----
            oh2 = sb.tile([BS, C], f32)    # one-hot * C/(C-1)
            nc.vector.tensor_scalar(out=oh2[:, :], in0=io_f[:, :], scalar1=ylab[:, 0:1],
                                    scalar2=float(C) / (C - 1), op0=ALU.is_equal,
                                    op1=ALU.mult)
            lv = sb.tile([BS, C], f32)
            nc.vector.tensor_tensor(out=lv[:, :], in0=dis[:, :], in1=oh2[:, :], op=ALU.mult)
            intraC = sb.tile([BS, 1], f32)
            nc.vector.tensor_reduce(out=intraC[:, :], in_=lv[:, :], axis=AX.X, op=ALU.add)
            loss = sb.tile([BS, 1], f32)
            nc.vector.tensor_scalar(out=loss[:, :], in0=rowsum[:, :], scalar1=-1.0 / (C - 1),
                                    scalar2=intraC[:, 0:1], op0=ALU.mult, op1=ALU.add)
            nc.sync.dma_start(loss_d[:, 0:1], loss[:, :])

    nc.finalize()
    return nc


def _get_nc():
    if "nc" not in _cache:
        _cache["nc"] = _build()
    return _cache["nc"]


def _host_precompute(feature_center):
    fc = np.asarray(feature_center, dtype=np.float64)
    g = fc.reshape(C, POOL, G).mean(axis=2)                  # [100, 8]
    yn = g / (np.linalg.norm(g, axis=1, keepdims=True) + 1e-6)
    ybar = yn.mean(axis=0)
    z = yn - ybar
    A = (z.T @ z) / (2 * C - 1)
    M = np.linalg.inv(A)
    M = 0.5 * (M + M.T)
    r = yn @ M                                               # [100, 8]  M y_c
    c0 = M @ ybar
    k2 = np.einsum('cd,ce,de->c', z, z, M)                   # z_c M z_c

    cp = np.zeros((POOL + 1, NCONST), dtype=np.float64)
    cp[0:POOL, 0:POOL] = M
    cp[0:POOL, 9] = ybar
    cp[0:POOL, 10:10 + C] = r.T
    cp[0:POOL, 10 + C:10 + 2 * C] = -2.0 * r.T
    cp[POOL, 10 + C:10 + 2 * C] = k2
    cp[0:POOL, 10 + 2 * C] = c0
    return cp.astype(np.float32)


def kernel(hidden, feature_center, y):
    from concourse import bass_utils

    hidden = np.ascontiguousarray(np.asarray(hidden, dtype=np.float32))
    yl = np.asarray(y).astype(np.float32).reshape(B, 1)
    cp = _host_precompute(feature_center)

    nc = _get_nc()
    in_maps = []
    for c in range(NCORES):
        in_maps.append({
            "hidden_in": hidden[c * BS:(c + 1) * BS],
            "const_in": cp,
            "ylab_in": np.ascontiguousarray(yl[c * BS:(c + 1) * BS]),
        })
    res = bass_utils.run_bass_kernel_spmd(nc, in_maps, core_ids=list(range(NCORES)))
    loss = np.concatenate([r["loss_out"][:, 0] for r in res.results])
    return np.float32(loss.mean())


# revision 9
# speedup vs baseline: 1.1023x; 1.1023x over previous
"""Trainium2 Bass kernel for nn_ContrastiveCenterLoss_M.

Math reduction
--------------
reference computes, per sample b and class c, a Mahalanobis distance between
the pooled-normalized hidden vector x_b (8-dim) and pooled-normalized class
center y_c (8-dim), where the 8x8 covariance is over the 200 points
{x_b (repeated 100x), y_0..y_99}:

    cov_b = A + beta d_b d_b^T,  A = S_y/199,  d_b = x_b - ybar,  beta = 50/199

A depends only on feature_center and is well-conditioned (cond ~1.9), so
pinv == inv and Sherman-Morrison collapses the per-sample pinv to a rank-1
correction of the shared M = inv(A).  Working in u = x - ybar coordinates
(all class-only terms folded into host constants):

    ur_c  = u.(M y_c)                     [one 9x128^T @ 9x201 matmul]
    e0    = u.(M ybar)                    [extra matmul column]
    uw    = u.M.u = x.w - e0,  w = M u    [small matmul + elementwise]
    gamma = 1/(1/beta + uw),   sg = sqrt(gamma)
    m[b,c] = (uw + 2 e0) + (k2_c - 2 ur_c) - (sg*ur_c - sg*(uw+e0))^2
    k2_c  = (y_c-ybar).M.(y_c-ybar)       [host]
    dis = sqrt(m);  loss_b = (C*dis[b,y_b] - sum_c dis[b,c])/(C-1)

Host precomputes the tiny center-only constants in float64; the device does
all per-sample work.  Data-parallel over batch: 8 cores x 128 samples.
ACT-table sqrt measured at ~1e-6 rel on HW, so no Newton refinement.
NOTE: InstTensorTensorReduce and [p,1]-shaped DRAM outputs crash the exec
unit on this runtime -- avoided (tt+reduce pairs; [128,100] output).
"""

import sys

if "/opt/trn_rl_repo" not in sys.path:
    sys.path.insert(0, "/opt/trn_rl_repo")

import numpy as np

B = 1024
D = 512
C = 100
POOL = 8
G = D // POOL          # 64
NCORES = 8
BS = B // NCORES       # 128 samples per core
BETA = (C / 2) / (2 * C - 1)   # 50/199
NCONST = 10 + 201      # [M(8x8)+pad | ybar9 | rhsU(9x201)] packed columns

_cache = {}


def _build():
    import concourse.mybir as mybir
    import concourse.tile as tile
    from concourse import bacc
    from concourse.masks import make_identity

    f32 = mybir.dt.float32
    ALU = mybir.AluOpType
    ACT = mybir.ActivationFunctionType
    AX = mybir.AxisListType
    HALF = D // 2

    nc = bacc.Bacc(
        "TRN2",
        target_bir_lowering=False,
        debug=False,
        enable_asserts=False,
        num_devices=NCORES,
    )

    hidden_d = nc.dram_tensor("hidden_in", [BS, D + 1], f32, kind="ExternalInput")
    const_d = nc.dram_tensor("const_in", [POOL + 1, NCONST], f32, kind="ExternalInput")
    loss_d = nc.dram_tensor("loss_out", [BS, C], f32, kind="ExternalOutput")

    with tile.TileContext(nc) as tc:
        with (
            tc.tile_pool(name="sb", bufs=1) as sb,
            tc.tile_pool(name="ps", bufs=1, space="PSUM") as ps,
        ):
            # ACT-table ordering hint: make the first ACT op a Sqrt so walrus
            # loads the sqrt set (which also contains square) exactly once,
            # early, overlapped with the DMA.
            warm = sb.tile([1, 1], f32)
            nc.vector.memset(warm[:, :], 1.0)
            nc.scalar.sqrt(out=warm[:, :], in_=warm[:, :])

            # one DMA for hidden + labels (y packed as f32 col 512)
            H = sb.tile([BS, D + 1], f32)
            nc.sync.dma_start(H[:, :], hidden_d[:, :])
            ylab = H[:, D:D + 1]
            cst = sb.tile([POOL + 1, NCONST], f32)
            nc.sync.dma_start(cst[:, :], const_d[:, :])
            msb = cst[0:POOL, 0:POOL]
            ybar9 = cst[:, 9:10]
            rhsU = cst[:, 10:10 + 201]

            # constants with no deps: identity (PE transpose) + iota (one-hot)
            ident = sb.tile([BS, BS], f32)
            make_identity(nc, ident[:, :])
            io_f = sb.tile([BS, C], f32)
            nc.gpsimd.iota(out=io_f[:, :], pattern=[[1, C]], base=0,
                           channel_multiplier=0, allow_small_or_imprecise_dtypes=True)

            # ---- pool hidden into 8 groups of 64, L2-normalize -> x ----
            s8 = sb.tile([BS, POOL], f32)
            nc.vector.tensor_reduce(
                out=s8[:, :],
                in_=H[:, 0:D].rearrange("p (g e) -> p g e", e=G),
                axis=AX.X, op=ALU.add,
            )
            sq = sb.tile([BS, POOL], f32)
            ss = sb.tile([BS, 1], f32)
            nc.scalar.activation(
                out=sq[:, :], in_=s8[:, :], func=ACT.Square, scale=1.0 / G,
                accum_out=ss[:, :],
            )
            nv = sb.tile([BS, 1], f32)
            nc.scalar.sqrt(out=nv[:, :], in_=ss[:, :])
            ne = sb.tile([BS, 1], f32)
            nc.vector.tensor_scalar(out=ne[:, :], in0=nv[:, :], scalar1=1e-6,
                                    scalar2=None, op0=ALU.add)
            rn = sb.tile([BS, 1], f32)
            nc.vector.reciprocal(out=rn[:, :], in_=ne[:, :])
            xn9 = sb.tile([BS, POOL + 1], f32)   # [x | 1]
            nc.vector.tensor_scalar(
                out=xn9[:, 0:POOL], in0=s8[:, :], scalar1=1.0 / G, scalar2=rn[:, 0:1],
                op0=ALU.mult, op1=ALU.mult,
            )
            nc.vector.memset(xn9[:, POOL:POOL + 1], 1.0)

            # ---- u^T (+ones row) = transpose(x|1) - (ybar|0) ----
            xnt_ps = ps.tile([POOL + 1, BS], f32)
            nc.tensor.transpose(xnt_ps[:, :], xn9[:, :], ident[:, :])
            ut9 = sb.tile([POOL + 1, BS], f32)
            nc.vector.tensor_scalar(
                out=ut9[:, :], in0=xnt_ps[:, :], scalar1=ybar9, scalar2=None,
                op0=ALU.subtract,
            )

            # ---- w = M u ; back to [128,8] ----
            wt_ps = ps.tile([POOL, BS], f32)
            nc.tensor.matmul(wt_ps[:, :], msb, ut9[0:POOL, :])
            lhsw = sb.tile([POOL, BS], f32)
            nc.scalar.copy(out=lhsw[:, :], in_=wt_ps[:, :])
            w128_ps = ps.tile([BS, POOL], f32)
            nc.tensor.transpose(w128_ps[:, :], lhsw[:, :], ident[0:POOL, 0:POOL])

            # ---- the one big contraction ----
            # cols 0:100 = u.r_c ; 100:200 = k2_c - 2 u.r_c ; 200 = e0 = u.(M ybar)
            dis_ps = ps.tile([BS, 201], f32)
            nc.tensor.matmul(dis_ps[:, :], ut9[:, :], rhsU)

            # ---- per-sample scalars (note uw + e0 = u.M.x = x.w = xws) ----
            xw = sb.tile([BS, POOL], f32)
            nc.vector.tensor_tensor(out=xw[:, :], in0=xn9[:, 0:POOL],
                                    in1=w128_ps[:, :], op=ALU.mult)
            xws = sb.tile([BS, 1], f32)
            nc.vector.tensor_reduce(out=xws[:, :], in_=xw[:, :], axis=AX.X, op=ALU.add)
            e0 = sb.tile([BS, 1], f32)
            nc.vector.tensor_copy(out=e0[:, :], in_=dis_ps[:, 200:201])
            den = sb.tile([BS, 1], f32)    # 1/beta + uw = (xws - e0) + 1/beta
            nc.vector.tensor_scalar(out=den[:, :], in0=xws[:, :], scalar1=e0[:, 0:1],
                                    scalar2=1.0 / BETA, op0=ALU.subtract, op1=ALU.add)
            gam = sb.tile([BS, 1], f32)
            nc.vector.reciprocal(out=gam[:, :], in_=den[:, :])
            sg = sb.tile([BS, 1], f32)
            nc.scalar.sqrt(out=sg[:, :], in_=gam[:, :])
            s2 = sb.tile([BS, 1], f32)     # uw + 2 e0 = xws + e0
            nc.vector.tensor_scalar(out=s2[:, :], in0=xws[:, :], scalar1=e0[:, 0:1],
                                    scalar2=None, op0=ALU.add)
            bneg = sb.tile([BS, 1], f32)   # -sg*(uw+e0) = -sg*xws
            nc.vector.tensor_scalar(out=bneg[:, :], in0=xws[:, :], scalar1=sg[:, 0:1],
                                    scalar2=-1.0, op0=ALU.mult, op1=ALU.mult)

            # ---- m = (k2 - 2ur + s2) - (sg*ur + bneg)^2 ; dis = sqrt(m) ----
            zsq = sb.tile([BS, C], f32)
            nc.scalar.activation(out=zsq[:, :], in_=dis_ps[:, 0:C], func=ACT.Square,
                                 scale=sg[:, 0:1], bias=bneg[:, 0:1])
            d1z = sb.tile([BS, C], f32)
            nc.vector.tensor_tensor(out=d1z[:, :], in0=dis_ps[:, C:2 * C],
                                    in1=zsq[:, :], op=ALU.subtract)
            dis = sb.tile([BS, C], f32)
            rowsum = sb.tile([BS, 1], f32)
            nc.scalar.activation(out=dis[:, :], in_=d1z[:, :], func=ACT.Sqrt,
                                 bias=s2[:, 0:1], accum_out=rowsum[:, :])

            # ---- loss_b = (C/(C-1)) dis[b,y_b] - rowsum/(C-1) ----
            oh2 = sb.tile([BS, C], f32)    # one-hot * C/(C-1)
            nc.vector.tensor_scalar(out=oh2[:, :], in0=io_f[:, :], scalar1=ylab[:, 0:1],
                                    scalar2=float(C) / (C - 1), op0=ALU.is_equal,
                                    op1=ALU.mult)
            lv = sb.tile([BS, C], f32)
            nc.vector.tensor_tensor(out=lv[:, :], in0=dis[:, :], in1=oh2[:, :], op=ALU.mult)
            intraC = sb.tile([BS, 1], f32)
            nc.vector.tensor_reduce(out=intraC[:, :], in_=lv[:, :], axis=AX.X, op=ALU.add)
            loss = sb.tile([BS, 1], f32)
            nc.vector.tensor_scalar(out=loss[:, :], in0=rowsum[:, :], scalar1=-1.0 / (C - 1),
                                    scalar2=intraC[:, 0:1], op0=ALU.mult, op1=ALU.add)
            nc.sync.dma_start(loss_d[:, 0:1], loss[:, :])

    nc.finalize()
    return nc


def _get_nc():
    if "nc" not in _cache:
        _cache["nc"] = _build()
    return _cache["nc"]


def _host_precompute(feature_center):
    fc = np.asarray(feature_center, dtype=np.float64)
    g = fc.reshape(C, POOL, G).mean(axis=2)                  # [100, 8]
    yn = g / (np.linalg.norm(g, axis=1, keepdims=True) + 1e-6)
    ybar = yn.mean(axis=0)
    z = yn - ybar
    A = (z.T @ z) / (2 * C - 1)
    M = np.linalg.inv(A)
    M = 0.5 * (M + M.T)
    r = yn @ M                                               # [100, 8]  M y_c
    c0 = M @ ybar
    k2 = np.einsum('cd,ce,de->c', z, z, M)                   # z_c M z_c

    cp = np.zeros((POOL + 1, NCONST), dtype=np.float64)
    cp[0:POOL, 0:POOL] = M
    cp[0:POOL, 9] = ybar
    cp[0:POOL, 10:10 + C] = r.T
    cp[0:POOL, 10 + C:10 + 2 * C] = -2.0 * r.T
    cp[POOL, 10 + C:10 + 2 * C] = k2
    cp[0:POOL, 10 + 2 * C] = c0
    return cp.astype(np.float32)


def kernel(hidden, feature_center, y):
    from concourse import bass_utils

    ha = np.empty((B, D + 1), dtype=np.float32)
    ha[:, 0:D] = np.asarray(hidden, dtype=np.float32)
    ha[:, D] = np.asarray(y).astype(np.float32)
    cp = _host_precompute(feature_center)

    nc = _get_nc()
    in_maps = []
    for c in range(NCORES):
        in_maps.append({
            "hidden_in": ha[c * BS:(c + 1) * BS],
            "const_in": cp,
        })
    res = bass_utils.run_bass_kernel_spmd(nc, in_maps, core_ids=list(range(NCORES)))
    loss = np.concatenate([r["loss_out"][:, 0] for r in res.results])
    return np.float32(loss.mean())


# revision 10
# speedup vs baseline: 1.1445x; 1.0382x over previous
"""Trainium2 Bass kernel for nn_ContrastiveCenterLoss_M.

Math reduction
--------------
reference computes, per sample b and class c, a Mahalanobis distance between
the pooled-normalized hidden vector x_b (8-dim) and pooled-normalized class
center y_c (8-dim), where the 8x8 covariance is over the 200 points
{x_b (repeated 100x), y_0..y_99}:

    cov_b = A + beta d_b d_b^T,  A = S_y/199,  d_b = x_b - ybar,  beta = 50/199

A depends only on feature_center and is well-conditioned (cond ~1.9), so
pinv == inv and Sherman-Morrison collapses the per-sample pinv to a rank-1
correction of the shared M = inv(A).  Working in u = x - ybar coordinates
(all class-only terms folded into host constants):

    ur_c  = u.(M y_c)                     [one 9x128^T @ 9x201 matmul]
    e0    = u.(M ybar)                    [extra matmul column]
    uw    = u.M.u = x.w - e0,  w = M u    [small matmul + elementwise]
    gamma = 1/(1/beta + uw),   sg = sqrt(gamma)
    m[b,c] = (uw + 2 e0) + (k2_c - 2 ur_c) - (sg*ur_c - sg*(uw+e0))^2
    k2_c  = (y_c-ybar).M.(y_c-ybar)       [host]
    dis = sqrt(m);  loss_b = (C*dis[b,y_b] - sum_c dis[b,c])/(C-1)

Host precomputes the tiny center-only constants in float64; the device does
all per-sample work.  Data-parallel over batch: 8 cores x 128 samples.
ACT-table sqrt measured at ~1e-6 rel on HW, so no Newton refinement.
NOTE: InstTensorTensorReduce and [p,1]-shaped DRAM outputs crash the exec
unit on this runtime -- avoided (tt+reduce pairs; [128,100] output).
"""

import sys

if "/opt/trn_rl_repo" not in sys.path:
    sys.path.insert(0, "/opt/trn_rl_repo")

import numpy as np

B = 1024
D = 512
C = 100
POOL = 8
G = D // POOL          # 64
NCORES = 8
BS = B // NCORES       # 128 samples per core
BETA = (C / 2) / (2 * C - 1)   # 50/199
NCONST = 1 + 209       # [ybar9 | rhsU(9x209)] packed columns

_cache = {}


def _build():
    import concourse.mybir as mybir
    import concourse.tile as tile
    from concourse import bacc
    from concourse.masks import make_identity

    f32 = mybir.dt.float32
    ALU = mybir.AluOpType
    ACT = mybir.ActivationFunctionType
    AX = mybir.AxisListType
    HALF = D // 2

    nc = bacc.Bacc(
        "TRN2",
        target_bir_lowering=False,
        debug=False,
        enable_asserts=False,
        num_devices=NCORES,
    )

    hidden_d = nc.dram_tensor("hidden_in", [BS, D + 1], f32, kind="ExternalInput")
    const_d = nc.dram_tensor("const_in", [POOL + 1, NCONST], f32, kind="ExternalInput")
    loss_d = nc.dram_tensor("loss_out", [BS, C], f32, kind="ExternalOutput")

    with tile.TileContext(nc) as tc:
        with (
            tc.tile_pool(name="sb", bufs=1) as sb,
            tc.tile_pool(name="ps", bufs=1, space="PSUM") as ps,
        ):
            # ACT-table ordering hint: make the first ACT op a Sqrt so walrus
            # loads the sqrt set (which also contains square) exactly once,
            # early, overlapped with the DMA.
            warm = sb.tile([1, 1], f32)
            nc.vector.memset(warm[:, :], 1.0)
            nc.scalar.sqrt(out=warm[:, :], in_=warm[:, :])

            # hidden + labels (y packed as f32 col 512), split for DMA/compute overlap
            h1 = sb.tile([BS, HALF], f32)
            h2 = sb.tile([BS, HALF + 1], f32)
            nc.sync.dma_start(h1[:, :], hidden_d[:, 0:HALF])
            nc.sync.dma_start(h2[:, :], hidden_d[:, HALF:D + 1])
            ylab = h2[:, HALF:HALF + 1]
            cst = sb.tile([POOL + 1, NCONST], f32)
            nc.sync.dma_start(cst[:, :], const_d[:, :])
            ybar9 = cst[:, 0:1]
            rhsU = cst[:, 1:1 + 209]

            # constants with no deps: identity (PE transpose) + iota (one-hot)
            ident = sb.tile([BS, BS], f32)
            make_identity(nc, ident[:, :])
            io_f = sb.tile([BS, C], f32)
            nc.gpsimd.iota(out=io_f[:, :], pattern=[[1, C]], base=0,
                           channel_multiplier=0, allow_small_or_imprecise_dtypes=True)

            # one-hot coefficient (C*onehot - 1)/(C-1): off the critical path
            oh = sb.tile([BS, C], f32)
            nc.vector.tensor_scalar(out=oh[:, :], in0=io_f[:, :], scalar1=ylab[:, 0:1],
                                    scalar2=float(C) / (C - 1), op0=ALU.is_equal,
                                    op1=ALU.mult)
            coef = sb.tile([BS, C], f32)
            nc.vector.tensor_scalar(out=coef[:, :], in0=oh[:, :], scalar1=-1.0 / (C - 1),
                                    scalar2=None, op0=ALU.add)

            # ---- pool hidden into 8 groups of 64, L2-normalize -> x ----
            s8 = sb.tile([BS, POOL], f32)
            nc.vector.tensor_reduce(
                out=s8[:, 0:POOL // 2],
                in_=h1[:, :].rearrange("p (g e) -> p g e", e=G),
                axis=AX.X, op=ALU.add,
            )
            nc.vector.tensor_reduce(
                out=s8[:, POOL // 2:POOL],
                in_=h2[:, 0:HALF].rearrange("p (g e) -> p g e", e=G),
                axis=AX.X, op=ALU.add,
            )
            sq = sb.tile([BS, POOL], f32)
            ss = sb.tile([BS, 1], f32)
            nc.scalar.activation(
                out=sq[:, :], in_=s8[:, :], func=ACT.Square, scale=1.0 / G,
                accum_out=ss[:, :],
            )
            nv = sb.tile([BS, 1], f32)
            nc.scalar.sqrt(out=nv[:, :], in_=ss[:, :])
            ne = sb.tile([BS, 1], f32)
            nc.vector.tensor_scalar(out=ne[:, :], in0=nv[:, :], scalar1=1e-6,
                                    scalar2=None, op0=ALU.add)
            rn = sb.tile([BS, 1], f32)
            nc.vector.reciprocal(out=rn[:, :], in_=ne[:, :])
            xn9 = sb.tile([BS, POOL + 1], f32)   # [x | 1]
            nc.vector.tensor_scalar(
                out=xn9[:, 0:POOL], in0=s8[:, :], scalar1=1.0 / G, scalar2=rn[:, 0:1],
                op0=ALU.mult, op1=ALU.mult,
            )
            nc.vector.memset(xn9[:, POOL:POOL + 1], 1.0)

            # ---- u^T (+ones row) = transpose(x|1) - (ybar|0) ----
            xnt_ps = ps.tile([POOL + 1, BS], f32)
            nc.tensor.transpose(xnt_ps[:, :], xn9[:, :], ident[:, :])
            ut9 = sb.tile([POOL + 1, BS], f32)
            nc.vector.tensor_scalar(
                out=ut9[:, :], in0=xnt_ps[:, :], scalar1=ybar9, scalar2=None,
                op0=ALU.subtract,
            )

            # ---- the one big contraction ----
            # cols 0:100 = u.r_c ; 100:200 = k2_c - 2 u.r_c ; 200 = e0 = u.(M ybar)
            # cols 201:209 = w = M u  (in [sample, dim] layout, no transpose needed)
            dis_ps = ps.tile([BS, 209], f32)
            nc.tensor.matmul(dis_ps[:, :], ut9[:, :], rhsU)

            # ---- per-sample scalars (note uw + e0 = u.M.x = x.w = xws) ----
            xw = sb.tile([BS, POOL], f32)
            nc.vector.tensor_tensor(out=xw[:, :], in0=xn9[:, 0:POOL],
                                    in1=dis_ps[:, 201:209], op=ALU.mult)
            xws = sb.tile([BS, 1], f32)
            nc.vector.tensor_reduce(out=xws[:, :], in_=xw[:, :], axis=AX.X, op=ALU.add)
            nxws = sb.tile([BS, 1], f32)
            nc.vector.tensor_scalar(out=nxws[:, :], in0=xws[:, :], scalar1=-1.0,
                                    scalar2=None, op0=ALU.mult)
            e0 = sb.tile([BS, 1], f32)
            nc.vector.tensor_copy(out=e0[:, :], in_=dis_ps[:, 200:201])
            den = sb.tile([BS, 1], f32)    # 1/beta + uw = (xws - e0) + 1/beta
            nc.vector.tensor_scalar(out=den[:, :], in0=xws[:, :], scalar1=e0[:, 0:1],
                                    scalar2=1.0 / BETA, op0=ALU.subtract, op1=ALU.add)
            gam = sb.tile([BS, 1], f32)
            nc.vector.reciprocal(out=gam[:, :], in_=den[:, :])
            s2 = sb.tile([BS, 1], f32)     # uw + 2 e0 = xws + e0
            nc.vector.tensor_scalar(out=s2[:, :], in0=xws[:, :], scalar1=e0[:, 0:1],
                                    scalar2=None, op0=ALU.add)

            # ---- m = (k2 - 2ur + s2) - gam*(ur - xws)^2 ; dis = sqrt(m) ----
            qsq = sb.tile([BS, C], f32)
            nc.scalar.activation(out=qsq[:, :], in_=dis_ps[:, 0:C], func=ACT.Square,
                                 bias=nxws[:, 0:1])
            gq = sb.tile([BS, C], f32)
            nc.vector.tensor_scalar(out=gq[:, :], in0=qsq[:, :], scalar1=gam[:, 0:1],
                                    scalar2=None, op0=ALU.mult)
            d1z = sb.tile([BS, C], f32)
            nc.vector.tensor_tensor(out=d1z[:, :], in0=dis_ps[:, C:2 * C],
                                    in1=gq[:, :], op=ALU.subtract)
            dis = sb.tile([BS, C], f32)
            nc.scalar.activation(out=dis[:, :], in_=d1z[:, :], func=ACT.Sqrt,
                                 bias=s2[:, 0:1])

            # ---- loss_b = sum_c dis[b,c] * coef[b,c] ----
            lv = sb.tile([BS, C], f32)
            nc.vector.tensor_tensor(out=lv[:, :], in0=dis[:, :], in1=coef[:, :], op=ALU.mult)
            loss = sb.tile([BS, 1], f32)
            nc.vector.tensor_reduce(out=loss[:, :], in_=lv[:, :], axis=AX.X, op=ALU.add)
            nc.sync.dma_start(loss_d[:, 0:1], loss[:, :])

    nc.finalize()
    return nc


def _get_nc():
    if "nc" not in _cache:
        _cache["nc"] = _build()
    return _cache["nc"]


def _host_precompute(feature_center):
    fc = np.asarray(feature_center, dtype=np.float64)
    g = fc.reshape(C, POOL, G).mean(axis=2)                  # [100, 8]
    yn = g / (np.linalg.norm(g, axis=1, keepdims=True) + 1e-6)
    ybar = yn.mean(axis=0)
    z = yn - ybar
    A = (z.T @ z) / (2 * C - 1)
    M = np.linalg.inv(A)
    M = 0.5 * (M + M.T)
    r = yn @ M                                               # [100, 8]  M y_c
    c0 = M @ ybar
    k2 = np.einsum('cd,ce,de->c', z, z, M)                   # z_c M z_c

    cp = np.zeros((POOL + 1, NCONST), dtype=np.float64)
    cp[0:POOL, 0] = ybar
    cp[0:POOL, 1:1 + C] = r.T
    cp[0:POOL, 1 + C:1 + 2 * C] = -2.0 * r.T
    cp[POOL, 1 + C:1 + 2 * C] = k2
    cp[0:POOL, 1 + 2 * C] = c0
    cp[0:POOL, 1 + 2 * C + 1:1 + 2 * C + 1 + POOL] = M
    return cp.astype(np.float32)


def kernel(hidden, feature_center, y):
    from concourse import bass_utils

    ha = np.empty((B, D + 1), dtype=np.float32)
    ha[:, 0:D] = np.asarray(hidden, dtype=np.float32)
    ha[:, D] = np.asarray(y).astype(np.float32)
    cp = _host_precompute(feature_center)

    nc = _get_nc()
    in_maps = []
    for c in range(NCORES):
        in_maps.append({
            "hidden_in": ha[c * BS:(c + 1) * BS],
            "const_in": cp,
        })
    res = bass_utils.run_bass_kernel_spmd(nc, in_maps, core_ids=list(range(NCORES)))
    loss = np.concatenate([r["loss_out"][:, 0] for r in res.results])
    return np.float32(loss.mean())


# revision 12
# speedup vs baseline: 1.1681x; 1.0206x over previous
"""Trainium2 Bass kernel for nn_ContrastiveCenterLoss_M.

Math reduction
--------------
reference computes, per sample b and class c, a Mahalanobis distance between
the pooled-normalized hidden vector x_b (8-dim) and pooled-normalized class
center y_c (8-dim), where the 8x8 covariance is over the 200 points
{x_b (repeated 100x), y_0..y_99}:

    cov_b = A + beta d_b d_b^T,  A = S_y/199,  d_b = x_b - ybar,  beta = 50/199

A depends only on feature_center and is well-conditioned (cond ~1.9), so
pinv == inv and Sherman-Morrison collapses the per-sample pinv to a rank-1
correction of the shared M = inv(A).  Working in u = x - ybar coordinates
(all class-only terms folded into host constants):

    ur_c  = u.(M y_c)                     [one 9x128^T @ 9x201 matmul]
    e0    = u.(M ybar)                    [extra matmul column]
    uw    = u.M.u = x.w - e0,  w = M u    [small matmul + elementwise]
    gamma = 1/(1/beta + uw),   sg = sqrt(gamma)
    m[b,c] = (uw + 2 e0) + (k2_c - 2 ur_c) - (sg*ur_c - sg*(uw+e0))^2
    k2_c  = (y_c-ybar).M.(y_c-ybar)       [host]
    dis = sqrt(m);  loss_b = (C*dis[b,y_b] - sum_c dis[b,c])/(C-1)

Host precomputes the tiny center-only constants in float64; the device does
all per-sample work.  Data-parallel over batch: 8 cores x 128 samples.
ACT-table sqrt measured at ~1e-6 rel on HW, so no Newton refinement.
NOTE: InstTensorTensorReduce and [p,1]-shaped DRAM outputs crash the exec
unit on this runtime -- avoided (tt+reduce pairs; [128,100] output).
"""

import sys

if "/opt/trn_rl_repo" not in sys.path:
    sys.path.insert(0, "/opt/trn_rl_repo")

import numpy as np

B = 1024
D = 512
C = 100
POOL = 8
G = D // POOL          # 64
NCORES = 8
BS = B // NCORES       # 128 samples per core
BETA = (C / 2) / (2 * C - 1)   # 50/199
NCONST = 1 + 209       # [ybar9 | rhsU(9x209)] packed columns

_cache = {}


def _build():
    import concourse.mybir as mybir
    import concourse.tile as tile
    from concourse import bacc
    from concourse.masks import make_identity

    f32 = mybir.dt.float32
    ALU = mybir.AluOpType
    ACT = mybir.ActivationFunctionType
    AX = mybir.AxisListType
    HALF = D // 2

    nc = bacc.Bacc(
        "TRN2",
        target_bir_lowering=False,
        debug=False,
        enable_asserts=False,
        num_devices=NCORES,
    )

    hidden_d = nc.dram_tensor("hidden_in", [BS, D + 1], f32, kind="ExternalInput")
    const_d = nc.dram_tensor("const_in", [POOL + 1, NCONST], f32, kind="ExternalInput")
    loss_d = nc.dram_tensor("loss_out", [BS, C], f32, kind="ExternalOutput")

    with tile.TileContext(nc) as tc:
        with (
            tc.tile_pool(name="sb", bufs=1) as sb,
            tc.tile_pool(name="ps", bufs=1, space="PSUM") as ps,
        ):
            # ACT-table ordering hint: make the first ACT op a Sqrt so walrus
            # loads the sqrt set (which also contains square) exactly once,
            # early, overlapped with the DMA.
            warm = sb.tile([1, 1], f32)
            nc.vector.memset(warm[:, :], 1.0)
            nc.scalar.sqrt(out=warm[:, :], in_=warm[:, :])

            # hidden + labels (y packed as f32 col 512), split for DMA/compute overlap
            h1 = sb.tile([BS, HALF], f32)
            h2 = sb.tile([BS, HALF + 1], f32)
            nc.sync.dma_start(h1[:, :], hidden_d[:, 0:HALF])
            nc.sync.dma_start(h2[:, :], hidden_d[:, HALF:D + 1])
            ylab = h2[:, HALF:HALF + 1]
            cst = sb.tile([POOL + 1, NCONST], f32)
            nc.sync.dma_start(cst[:, :], const_d[:, :])
            ybar9 = cst[:, 0:1]
            rhsU = cst[:, 1:1 + 209]

            # constants with no deps: identity (PE transpose) + iota (one-hot)
            ident = sb.tile([BS, BS], f32)
            make_identity(nc, ident[:, :])
            io_f = sb.tile([BS, C], f32)
            nc.gpsimd.iota(out=io_f[:, :], pattern=[[1, C]], base=0,
                           channel_multiplier=0, allow_small_or_imprecise_dtypes=True)

            # ---- pool hidden into 8 groups of 64, L2-normalize -> x ----
            s8 = sb.tile([BS, POOL], f32)
            nc.vector.tensor_reduce(
                out=s8[:, 0:POOL // 2],
                in_=h1[:, :].rearrange("p (g e) -> p g e", e=G),
                axis=AX.X, op=ALU.add,
            )
            nc.vector.tensor_reduce(
                out=s8[:, POOL // 2:POOL],
                in_=h2[:, 0:HALF].rearrange("p (g e) -> p g e", e=G),
                axis=AX.X, op=ALU.add,
            )
            sq = sb.tile([BS, POOL], f32)
            ss = sb.tile([BS, 1], f32)
            nc.scalar.activation(
                out=sq[:, :], in_=s8[:, :], func=ACT.Square, scale=1.0 / G,
                accum_out=ss[:, :],
            )
            nv = sb.tile([BS, 1], f32)
            nc.scalar.sqrt(out=nv[:, :], in_=ss[:, :])
            ne = sb.tile([BS, 1], f32)
            nc.vector.tensor_scalar(out=ne[:, :], in0=nv[:, :], scalar1=1e-6,
                                    scalar2=None, op0=ALU.add)
            rn = sb.tile([BS, 1], f32)
            nc.vector.reciprocal(out=rn[:, :], in_=ne[:, :])
            xn9 = sb.tile([BS, POOL + 1], f32)   # [x | 1]
            nc.vector.tensor_scalar(
                out=xn9[:, 0:POOL], in0=s8[:, :], scalar1=1.0 / G, scalar2=rn[:, 0:1],
                op0=ALU.mult, op1=ALU.mult,
            )
            nc.vector.memset(xn9[:, POOL:POOL + 1], 1.0)

            # one-hot coefficient (C*onehot - 1)/(C-1): off the critical path
            oh = sb.tile([BS, C], f32)
            nc.vector.tensor_scalar(out=oh[:, :], in0=io_f[:, :], scalar1=ylab[:, 0:1],
                                    scalar2=float(C) / (C - 1), op0=ALU.is_equal,
                                    op1=ALU.mult)
            coef = sb.tile([BS, C], f32)
            nc.vector.tensor_scalar(out=coef[:, :], in0=oh[:, :], scalar1=-1.0 / (C - 1),
                                    scalar2=None, op0=ALU.add)

            # ---- u^T (+ones row) = transpose(x|1) - (ybar|0) ----
            xnt_ps = ps.tile([POOL + 1, BS], f32)
            nc.tensor.transpose(xnt_ps[:, :], xn9[:, :], ident[:, :])
            ut9 = sb.tile([POOL + 1, BS], f32)
            nc.vector.tensor_scalar(
                out=ut9[:, :], in0=xnt_ps[:, :], scalar1=ybar9, scalar2=None,
                op0=ALU.subtract,
            )

            # ---- the one big contraction ----
            # cols 0:100 = u.r_c ; 100:200 = k2_c - 2 u.r_c ; 200 = e0 = u.(M ybar)
            # cols 201:209 = w = M u  (in [sample, dim] layout, no transpose needed)
            dis_ps = ps.tile([BS, 209], f32)
            nc.tensor.matmul(dis_ps[:, :], ut9[:, :], rhsU)

            # ---- per-sample scalars (note uw + e0 = u.M.x = x.w = xws) ----
            xw = sb.tile([BS, POOL], f32)
            nc.vector.tensor_tensor(out=xw[:, :], in0=xn9[:, 0:POOL],
                                    in1=dis_ps[:, 201:209], op=ALU.mult)
            xws = sb.tile([BS, 1], f32)
            nc.vector.tensor_reduce(out=xws[:, :], in_=xw[:, :], axis=AX.X, op=ALU.add)
            e0 = sb.tile([BS, 1], f32)
            nc.vector.tensor_copy(out=e0[:, :], in_=dis_ps[:, 200:201])
            den = sb.tile([BS, 1], f32)    # 1/beta + uw = (xws - e0) + 1/beta
            nc.vector.tensor_scalar(out=den[:, :], in0=xws[:, :], scalar1=e0[:, 0:1],
                                    scalar2=1.0 / BETA, op0=ALU.subtract, op1=ALU.add)
            gam = sb.tile([BS, 1], f32)
            nc.vector.reciprocal(out=gam[:, :], in_=den[:, :])
            s2 = sb.tile([BS, 1], f32)     # uw + 2 e0 = xws + e0
            nc.vector.tensor_scalar(out=s2[:, :], in0=xws[:, :], scalar1=e0[:, 0:1],
                                    scalar2=None, op0=ALU.add)

            # ---- m = (k2 - 2ur + s2) - gam*(ur - xws)^2 ; dis = sqrt(m) ----
            q = sb.tile([BS, C], f32)
            nc.vector.tensor_scalar(out=q[:, :], in0=dis_ps[:, 0:C], scalar1=xws[:, 0:1],
                                    scalar2=None, op0=ALU.subtract)
            qsq = sb.tile([BS, C], f32)
            nc.vector.tensor_tensor(out=qsq[:, :], in0=q[:, :], in1=q[:, :], op=ALU.mult)
            gq = sb.tile([BS, C], f32)
            nc.vector.tensor_scalar(out=gq[:, :], in0=qsq[:, :], scalar1=gam[:, 0:1],
                                    scalar2=None, op0=ALU.mult)
            d1z = sb.tile([BS, C], f32)
            nc.vector.tensor_tensor(out=d1z[:, :], in0=dis_ps[:, C:2 * C],
                                    in1=gq[:, :], op=ALU.subtract)
            dis = sb.tile([BS, C], f32)
            nc.scalar.activation(out=dis[:, :], in_=d1z[:, :], func=ACT.Sqrt,
                                 bias=s2[:, 0:1])

            # ---- loss_b = sum_c dis[b,c] * coef[b,c] ----
            lv = sb.tile([BS, C], f32)
            nc.vector.tensor_tensor(out=lv[:, :], in0=dis[:, :], in1=coef[:, :], op=ALU.mult)
            loss = sb.tile([BS, 1], f32)
            nc.vector.tensor_reduce(out=loss[:, :], in_=lv[:, :], axis=AX.X, op=ALU.add)
            nc.sync.dma_start(loss_d[:, 0:1], loss[:, :])

    nc.finalize()
    return nc


def _get_nc():
    if "nc" not in _cache:
        _cache["nc"] = _build()
    return _cache["nc"]


def _host_precompute(feature_center):
    fc = np.asarray(feature_center, dtype=np.float64)
    g = fc.reshape(C, POOL, G).mean(axis=2)                  # [100, 8]
    yn = g / (np.linalg.norm(g, axis=1, keepdims=True) + 1e-6)
    ybar = yn.mean(axis=0)
    z = yn - ybar
    A = (z.T @ z) / (2 * C - 1)
    M = np.linalg.inv(A)
    M = 0.5 * (M + M.T)
    r = yn @ M                                               # [100, 8]  M y_c
    c0 = M @ ybar
    k2 = np.einsum('cd,ce,de->c', z, z, M)                   # z_c M z_c

    cp = np.zeros((POOL + 1, NCONST), dtype=np.float64)
    cp[0:POOL, 0] = ybar
    cp[0:POOL, 1:1 + C] = r.T
    cp[0:POOL, 1 + C:1 + 2 * C] = -2.0 * r.T
    cp[POOL, 1 + C:1 + 2 * C] = k2
    cp[0:POOL, 1 + 2 * C] = c0
    cp[0:POOL, 1 + 2 * C + 1:1 + 2 * C + 1 + POOL] = M
    return cp.astype(np.float32)


def kernel(hidden, feature_center, y):
    from concourse import bass_utils

    ha = np.empty((B, D + 1), dtype=np.float32)
    ha[:, 0:D] = np.asarray(hidden, dtype=np.float32)
    ha[:, D] = np.asarray(y).astype(np.float32)
    cp = _host_precompute(feature_center)

    nc = _get_nc()
    in_maps = []
    for c in range(NCORES):
        in_maps.append({
            "hidden_in": ha[c * BS:(c + 1) * BS],
            "const_in": cp,
        })
    res = bass_utils.run_bass_kernel_spmd(nc, in_maps, core_ids=list(range(NCORES)))
    loss = np.concatenate([r["loss_out"][:, 0] for r in res.results])
    return np.float32(loss.mean())


# revision 14
# speedup vs baseline: 1.1947x; 1.0227x over previous
"""Trainium2 Bass kernel for nn_ContrastiveCenterLoss_M.

Math reduction
--------------
reference computes, per sample b and class c, a Mahalanobis distance between
the pooled-normalized hidden vector x_b (8-dim) and pooled-normalized class
center y_c (8-dim), where the 8x8 covariance is over the 200 points
{x_b (repeated 100x), y_0..y_99}:

    cov_b = A + beta d_b d_b^T,  A = S_y/199,  d_b = x_b - ybar,  beta = 50/199

A depends only on feature_center and is well-conditioned (cond ~1.9), so
pinv == inv and Sherman-Morrison collapses the per-sample pinv to a rank-1
correction of the shared M = inv(A).  Working in u = x - ybar coordinates
(all class-only terms folded into host constants):

    ur_c  = u.(M y_c)                     [one 9x128^T @ 9x209 matmul]
    e0    = u.(M ybar)                    [extra matmul column]
    uw    = u.M.u = x.w - e0,  w = M u    [w = 8 extra matmul columns]
    gamma = 1/(1/beta + uw),   sg = sqrt(gamma)
    m[b,c] = (uw + 2 e0) + (k2_c - 2 ur_c) - (sg*ur_c - sg*(uw+e0))^2
    k2_c  = (y_c-ybar).M.(y_c-ybar)       [host]
    dis = sqrt(m);  loss_b = (C*dis[b,y_b] - sum_c dis[b,c])/(C-1)

Host precomputes the tiny center-only constants in float64; the device does
all per-sample work.  Data-parallel over batch: 8 cores x 128 samples.
ACT-table sqrt measured at ~1e-6 rel on HW, so no Newton refinement.
NOTE: InstTensorTensorReduce and [p,1]-shaped DRAM outputs crash the exec
unit on this runtime -- avoided (tt+reduce pairs; [128,100] output).
"""

import sys

if "/opt/trn_rl_repo" not in sys.path:
    sys.path.insert(0, "/opt/trn_rl_repo")

import numpy as np

B = 1024
D = 512
C = 100
POOL = 8
G = D // POOL          # 64
NCORES = 8
BS = B // NCORES       # 128 samples per core
BETA = (C / 2) / (2 * C - 1)   # 50/199
NCONST = 1 + 209       # [ybar9 | rhsU(9x209)] packed columns

_cache = {}


def _build():
    import concourse.mybir as mybir
    import concourse.tile as tile
    from concourse import bacc
    from concourse.masks import make_identity

    f32 = mybir.dt.float32
    ALU = mybir.AluOpType
    ACT = mybir.ActivationFunctionType
    AX = mybir.AxisListType
    HALF = D // 2

    nc = bacc.Bacc(
        "TRN2",
        target_bir_lowering=False,
        debug=False,
        enable_asserts=False,
        num_devices=NCORES,
    )

    hidden_d = nc.dram_tensor("hidden_in", [BS, D + 1], f32, kind="ExternalInput")
    const_d = nc.dram_tensor("const_in", [POOL + 1, NCONST], f32, kind="ExternalInput")
    loss_d = nc.dram_tensor("loss_out", [BS, C], f32, kind="ExternalOutput")

    with tile.TileContext(nc) as tc:
        with (
            tc.tile_pool(name="sb", bufs=1) as sb,
            tc.tile_pool(name="ps", bufs=1, space="PSUM") as ps,
        ):
            # ACT-table ordering hint: make the first ACT op a Sqrt so walrus
            # loads the sqrt set (which also contains square) exactly once,
            # early, overlapped with the DMA.
            warm = sb.tile([1, 1], f32)
            nc.vector.memset(warm[:, :], 1.0)
            nc.scalar.sqrt(out=warm[:, :], in_=warm[:, :])

            # hidden + labels (y packed as f32 col 512), split for DMA/compute overlap
            h1 = sb.tile([BS, HALF], f32)
            h2 = sb.tile([BS, HALF + 1], f32)
            nc.sync.dma_start(h1[:, :], hidden_d[:, 0:HALF])
            nc.sync.dma_start(h2[:, :], hidden_d[:, HALF:D + 1])
            ylab = h2[:, HALF:HALF + 1]
            cst = sb.tile([POOL + 1, NCONST], f32)
            nc.sync.dma_start(cst[:, :], const_d[:, :])
            ybar9 = cst[:, 0:1]
            rhsU = cst[:, 1:1 + 209]

            # constants with no deps: identity (PE transpose) + iota (one-hot)
            ident = sb.tile([BS, BS], f32)
            make_identity(nc, ident[:, :])
            io_f = sb.tile([BS, C], f32)
            nc.gpsimd.iota(out=io_f[:, :], pattern=[[1, C]], base=0,
                           channel_multiplier=0, allow_small_or_imprecise_dtypes=True)

            # ---- pool hidden into 8 groups of 64, L2-normalize -> x ----
            s8 = sb.tile([BS, POOL], f32)
            nc.vector.tensor_reduce(
                out=s8[:, 0:POOL // 2],
                in_=h1[:, :].rearrange("p (g e) -> p g e", e=G),
                axis=AX.X, op=ALU.add,
            )
            nc.vector.tensor_reduce(
                out=s8[:, POOL // 2:POOL],
                in_=h2[:, 0:HALF].rearrange("p (g e) -> p g e", e=G),
                axis=AX.X, op=ALU.add,
            )
            sq = sb.tile([BS, POOL], f32)
            ss = sb.tile([BS, 1], f32)
            nc.scalar.activation(
                out=sq[:, :], in_=s8[:, :], func=ACT.Square, scale=1.0 / G,
                accum_out=ss[:, :],
            )
            nv = sb.tile([BS, 1], f32)
            nc.scalar.sqrt(out=nv[:, :], in_=ss[:, :])
            ne = sb.tile([BS, 1], f32)
            nc.vector.tensor_scalar(out=ne[:, :], in0=nv[:, :], scalar1=1e-6,
                                    scalar2=None, op0=ALU.add)
            rn = sb.tile([BS, 1], f32)
            nc.vector.reciprocal(out=rn[:, :], in_=ne[:, :])
            xn9 = sb.tile([BS, POOL + 1], f32)   # [x | 1]
            nc.vector.tensor_scalar(
                out=xn9[:, 0:POOL], in0=s8[:, :], scalar1=1.0 / G, scalar2=rn[:, 0:1],
                op0=ALU.mult, op1=ALU.mult,
            )
            nc.vector.memset(xn9[:, POOL:POOL + 1], 1.0)

            # one-hot mask of the true class: off the critical path
            oh = sb.tile([BS, C], f32)
            nc.vector.tensor_scalar(out=oh[:, :], in0=io_f[:, :], scalar1=ylab[:, 0:1],
                                    scalar2=None, op0=ALU.is_equal)

            # ---- u^T (+ones row) = transpose(x|1) - (ybar|0) ----
            xnt_ps = ps.tile([POOL + 1, BS], f32)
            nc.tensor.transpose(xnt_ps[:, :], xn9[:, :], ident[:, :])
            ut9 = sb.tile([POOL + 1, BS], f32)
            nc.vector.tensor_scalar(
                out=ut9[:, :], in0=xnt_ps[:, :], scalar1=ybar9, scalar2=None,
                op0=ALU.subtract,
            )

            # ---- the one big contraction ----
            # cols 0:100 = u.r_c ; 100:200 = k2_c - 2 u.r_c ; 200 = e0 = u.(M ybar)
            # cols 201:209 = w = M u  (in [sample, dim] layout, no transpose needed)
            dis_ps = ps.tile([BS, 209], f32)
            nc.tensor.matmul(dis_ps[:, :], ut9[:, :], rhsU)

            # ---- per-sample scalars (note uw + e0 = u.M.x = x.w = xws) ----
            xw = sb.tile([BS, POOL], f32)
            nc.vector.tensor_tensor(out=xw[:, :], in0=xn9[:, 0:POOL],
                                    in1=dis_ps[:, 201:209], op=ALU.mult)
            xws = sb.tile([BS, 1], f32)
            nc.vector.tensor_reduce(out=xws[:, :], in_=xw[:, :], axis=AX.X, op=ALU.add)
            e0 = sb.tile([BS, 1], f32)
            nc.vector.tensor_copy(out=e0[:, :], in_=dis_ps[:, 200:201])
            den = sb.tile([BS, 1], f32)    # 1/beta + uw = (xws - e0) + 1/beta
            nc.vector.tensor_scalar(out=den[:, :], in0=xws[:, :], scalar1=e0[:, 0:1],
                                    scalar2=1.0 / BETA, op0=ALU.subtract, op1=ALU.add)
            gam = sb.tile([BS, 1], f32)
            nc.vector.reciprocal(out=gam[:, :], in_=den[:, :])
            s2 = sb.tile([BS, 1], f32)     # uw + 2 e0 = xws + e0
            nc.vector.tensor_scalar(out=s2[:, :], in0=xws[:, :], scalar1=e0[:, 0:1],
                                    scalar2=None, op0=ALU.add)

            # ---- m = (k2 - 2ur) - (gam*(ur - xws)^2 - s2) ; dis = sqrt(m) ----
            nxws = sb.tile([BS, 1], f32)
            nc.vector.tensor_scalar(out=nxws[:, :], in0=xws[:, :], scalar1=-1.0,
                                    scalar2=None, op0=ALU.mult)
            qsq = sb.tile([BS, C], f32)     # (ur - xws)^2 on ACT, parallel with DVE
            nc.scalar.activation(out=qsq[:, :], in_=dis_ps[:, 0:C], func=ACT.Square,
                                 bias=nxws[:, 0:1])
            gq2 = sb.tile([BS, C], f32)     # gam*qsq - s2
            nc.vector.tensor_scalar(out=gq2[:, :], in0=qsq[:, :], scalar1=gam[:, 0:1],
                                    scalar2=s2[:, 0:1], op0=ALU.mult, op1=ALU.subtract)
            m = sb.tile([BS, C], f32)
            nc.vector.tensor_tensor(out=m[:, :], in0=dis_ps[:, C:2 * C],
                                    in1=gq2[:, :], op=ALU.subtract)

            # rowsum via ACT accum; true-class element via mask+reduce (parallel)
            dis = sb.tile([BS, C], f32)
            rowsum = sb.tile([BS, 1], f32)
            nc.scalar.activation(out=dis[:, :], in_=m[:, :], func=ACT.Sqrt,
                                 accum_out=rowsum[:, :])
            mh = sb.tile([BS, C], f32)
            nc.vector.tensor_tensor(out=mh[:, :], in0=m[:, :], in1=oh[:, :], op=ALU.mult)
            mt = sb.tile([BS, 1], f32)
            nc.vector.tensor_reduce(out=mt[:, :], in_=mh[:, :], axis=AX.X, op=ALU.add)
            intra = sb.tile([BS, 1], f32)
            nc.scalar.sqrt(out=intra[:, :], in_=mt[:, :])
            rs1 = sb.tile([BS, 1], f32)
            nc.vector.tensor_scalar(out=rs1[:, :], in0=rowsum[:, :], scalar1=-1.0 / (C - 1),
                                    scalar2=None, op0=ALU.mult)
            loss = sb.tile([BS, 1], f32)
            nc.vector.tensor_scalar(out=loss[:, :], in0=intra[:, :],
                                    scalar1=float(C) / (C - 1), scalar2=rs1[:, 0:1],
                                    op0=ALU.mult, op1=ALU.add)
            nc.sync.dma_start(loss_d[:, 0:1], loss[:, :])

    nc.finalize()
    return nc


def _get_nc():
    if "nc" not in _cache:
        _cache["nc"] = _build()
    return _cache["nc"]


def _host_precompute(feature_center):
    fc = np.asarray(feature_center, dtype=np.float64)
    g = fc.reshape(C, POOL, G).mean(axis=2)                  # [100, 8]
    yn = g / (np.linalg.norm(g, axis=1, keepdims=True) + 1e-6)
    ybar = yn.mean(axis=0)
    z = yn - ybar
    A = (z.T @ z) / (2 * C - 1)
    M = np.linalg.inv(A)
    M = 0.5 * (M + M.T)
    r = yn @ M                                               # [100, 8]  M y_c
    c0 = M @ ybar
    k2 = np.einsum('cd,ce,de->c', z, z, M)                   # z_c M z_c

    cp = np.zeros((POOL + 1, NCONST), dtype=np.float64)
    cp[0:POOL, 0] = ybar
    cp[0:POOL, 1:1 + C] = r.T
    cp[0:POOL, 1 + C:1 + 2 * C] = -2.0 * r.T
    cp[POOL, 1 + C:1 + 2 * C] = k2
    cp[0:POOL, 1 + 2 * C] = c0
    cp[0:POOL, 1 + 2 * C + 1:1 + 2 * C + 1 + POOL] = M
    return cp.astype(np.float32)


def kernel(hidden, feature_center, y):
    from concourse import bass_utils

    ha = np.empty((B, D + 1), dtype=np.float32)
    ha[:, 0:D] = np.asarray(hidden, dtype=np.float32)
    ha[:, D] = np.asarray(y).astype(np.float32)
    cp = _host_precompute(feature_center)

    nc = _get_nc()
    in_maps = []
    for c in range(NCORES):
        in_maps.append({
            "hidden_in": ha[c * BS:(c + 1) * BS],
            "const_in": cp,
        })
    res = bass_utils.run_bass_kernel_spmd(nc, in_maps, core_ids=list(range(NCORES)))
    loss = np.concatenate([r["loss_out"][:, 0] for r in res.results])
    return np.float32(loss.mean())


# revision 15
# speedup vs baseline: 1.2433x; 1.0408x over previous
"""Trainium2 Bass kernel for nn_ContrastiveCenterLoss_M.

Math reduction
--------------
reference computes, per sample b and class c, a Mahalanobis distance between
the pooled-normalized hidden vector x_b (8-dim) and pooled-normalized class
center y_c (8-dim), where the 8x8 covariance is over the 200 points
{x_b (repeated 100x), y_0..y_99}:

    cov_b = A + beta d_b d_b^T,  A = S_y/199,  d_b = x_b - ybar,  beta = 50/199

A depends only on feature_center and is well-conditioned (cond ~1.9), so
pinv == inv and Sherman-Morrison collapses the per-sample pinv to a rank-1
correction of the shared M = inv(A).  Working in u = x - ybar coordinates
(all class-only terms folded into host constants):

    ur_c  = u.(M y_c)                     [one 9x128^T @ 9x209 matmul]
    e0    = u.(M ybar)                    [extra matmul column]
    uw    = u.M.u = x.w - e0,  w = M u    [w = 8 extra matmul columns]
    gamma = 1/(1/beta + uw),   sg = sqrt(gamma)
    m[b,c] = (uw + 2 e0) + (k2_c - 2 ur_c) - (sg*ur_c - sg*(uw+e0))^2
    k2_c  = (y_c-ybar).M.(y_c-ybar)       [host]
    dis = sqrt(m);  loss_b = (C*dis[b,y_b] - sum_c dis[b,c])/(C-1)

Host precomputes the tiny center-only constants in float64; the device does
all per-sample work.  Data-parallel over batch: 8 cores x 128 samples.
ACT-table sqrt measured at ~1e-6 rel on HW, so no Newton refinement.
NOTE: InstTensorTensorReduce and [p,1]-shaped DRAM outputs crash the exec
unit on this runtime -- avoided (tt+reduce pairs; [128,100] output).
"""

import sys

if "/opt/trn_rl_repo" not in sys.path:
    sys.path.insert(0, "/opt/trn_rl_repo")

import numpy as np

B = 1024
D = 512
C = 100
POOL = 8
G = D // POOL          # 64
NCORES = 8
BS = B // NCORES       # 128 samples per core
BETA = (C / 2) / (2 * C - 1)   # 50/199
NCONST = 1 + 209       # [ybar9 | rhsU(9x209)] packed columns

_cache = {}


def _build():
    import concourse.mybir as mybir
    import concourse.tile as tile
    from concourse import bacc
    from concourse.masks import make_identity

    f32 = mybir.dt.float32
    ALU = mybir.AluOpType
    ACT = mybir.ActivationFunctionType
    AX = mybir.AxisListType
    HALF = D // 2

    nc = bacc.Bacc(
        "TRN2",
        target_bir_lowering=False,
        debug=False,
        enable_asserts=False,
        num_devices=NCORES,
    )

    hidden_d = nc.dram_tensor("hidden_in", [BS, D + 1], f32, kind="ExternalInput")
    const_d = nc.dram_tensor("const_in", [POOL + 1, NCONST], f32, kind="ExternalInput")
    loss_d = nc.dram_tensor("loss_out", [BS, C], f32, kind="ExternalOutput")

    with tile.TileContext(nc) as tc:
        with (
            tc.tile_pool(name="sb", bufs=1) as sb,
            tc.tile_pool(name="ps", bufs=1, space="PSUM") as ps,
        ):
            # ACT-table ordering hint: make the first ACT op a Sqrt so walrus
            # loads the sqrt set (which also contains square) exactly once,
            # early, overlapped with the DMA.
            warm = sb.tile([1, 1], f32)
            nc.vector.memset(warm[:, :], 1.0)
            nc.scalar.sqrt(out=warm[:, :], in_=warm[:, :])

            # hidden + labels (y packed as f32 col 512), split for DMA/compute overlap
            h1 = sb.tile([BS, HALF], f32)
            h2 = sb.tile([BS, HALF + 1], f32)
            nc.sync.dma_start(h1[:, :], hidden_d[:, 0:HALF])
            nc.sync.dma_start(h2[:, :], hidden_d[:, HALF:D + 1])
            ylab = h2[:, HALF:HALF + 1]
            cst = sb.tile([POOL + 1, NCONST], f32)
            nc.sync.dma_start(cst[:, :], const_d[:, :])
            ybar9 = cst[:, 0:1]
            rhsU = cst[:, 1:1 + 209]

            # constants with no deps: identity (PE transpose) + iota (one-hot)
            ident = sb.tile([BS, BS], f32)
            make_identity(nc, ident[:, :])
            io_f = sb.tile([BS, C], f32)
            nc.gpsimd.iota(out=io_f[:, :], pattern=[[1, C]], base=0,
                           channel_multiplier=0, allow_small_or_imprecise_dtypes=True)

            # ---- pool hidden into 8 groups of 64, L2-normalize -> x ----
            s8 = sb.tile([BS, POOL], f32)
            nc.vector.tensor_reduce(
                out=s8[:, 0:POOL // 2],
                in_=h1[:, :].rearrange("p (g e) -> p g e", e=G),
                axis=AX.X, op=ALU.add,
            )
            nc.vector.tensor_reduce(
                out=s8[:, POOL // 2:POOL],
                in_=h2[:, 0:HALF].rearrange("p (g e) -> p g e", e=G),
                axis=AX.X, op=ALU.add,
            )
            sq = sb.tile([BS, POOL], f32)
            ss = sb.tile([BS, 1], f32)
            nc.scalar.activation(
                out=sq[:, :], in_=s8[:, :], func=ACT.Square, scale=1.0 / G,
                accum_out=ss[:, :],
            )
            nv = sb.tile([BS, 1], f32)
            nc.scalar.sqrt(out=nv[:, :], in_=ss[:, :])
            ne = sb.tile([BS, 1], f32)
            nc.vector.tensor_scalar(out=ne[:, :], in0=nv[:, :], scalar1=1e-6,
                                    scalar2=None, op0=ALU.add)
            rn = sb.tile([BS, 1], f32)
            nc.vector.reciprocal(out=rn[:, :], in_=ne[:, :])
            xn9 = sb.tile([BS, POOL + 1], f32)   # [x | 1]
            nc.vector.tensor_scalar(
                out=xn9[:, 0:POOL], in0=s8[:, :], scalar1=1.0 / G, scalar2=rn[:, 0:1],
                op0=ALU.mult, op1=ALU.mult,
            )
            nc.vector.memset(xn9[:, POOL:POOL + 1], 1.0)

            # one-hot mask of the true class: off the critical path
            oh = sb.tile([BS, C], f32)
            nc.vector.tensor_scalar(out=oh[:, :], in0=io_f[:, :], scalar1=ylab[:, 0:1],
                                    scalar2=None, op0=ALU.is_equal)

            # ---- u^T (+ones row) = transpose(x|1) - (ybar|0) ----
            xnt_ps = ps.tile([POOL + 1, BS], f32)
            nc.tensor.transpose(xnt_ps[:, :], xn9[:, :], ident[:, :])
            ut9 = sb.tile([POOL + 1, BS], f32)
            nc.vector.tensor_scalar(
                out=ut9[:, :], in0=xnt_ps[:, :], scalar1=ybar9, scalar2=None,
                op0=ALU.subtract,
            )

            # ---- the contraction, split so the small w/e0 block lands first
            # and the per-sample scalar chain overlaps the big 200-col matmul.
            # wps col 0 = e0 = u.(M ybar) ; cols 1:9 = w = M u
            # dis_ps cols 0:100 = u.r_c ; 100:200 = k2_c - 2 u.r_c
            wps = ps.tile([BS, 9], f32)
            nc.tensor.matmul(wps[:, :], ut9[:, :], rhsU[:, 200:209])
            dis_ps = ps.tile([BS, 200], f32)
            nc.tensor.matmul(dis_ps[:, :], ut9[:, :], rhsU[:, 0:200])

            # ---- per-sample scalars (note uw + e0 = u.M.x = x.w = xws) ----
            xw = sb.tile([BS, POOL], f32)
            nc.vector.tensor_tensor(out=xw[:, :], in0=xn9[:, 0:POOL],
                                    in1=wps[:, 1:9], op=ALU.mult)
            xws = sb.tile([BS, 1], f32)
            nc.vector.tensor_reduce(out=xws[:, :], in_=xw[:, :], axis=AX.X, op=ALU.add)
            e0 = sb.tile([BS, 1], f32)
            nc.vector.tensor_copy(out=e0[:, :], in_=wps[:, 0:1])
            den = sb.tile([BS, 1], f32)    # 1/beta + uw = (xws - e0) + 1/beta
            nc.vector.tensor_scalar(out=den[:, :], in0=xws[:, :], scalar1=e0[:, 0:1],
                                    scalar2=1.0 / BETA, op0=ALU.subtract, op1=ALU.add)
            gam = sb.tile([BS, 1], f32)
            nc.vector.reciprocal(out=gam[:, :], in_=den[:, :])
            s2 = sb.tile([BS, 1], f32)     # uw + 2 e0 = xws + e0
            nc.vector.tensor_scalar(out=s2[:, :], in0=xws[:, :], scalar1=e0[:, 0:1],
                                    scalar2=None, op0=ALU.add)

            # ---- m = (k2 - 2ur) - (gam*(ur - xws)^2 - s2) ; dis = sqrt(m) ----
            nxws = sb.tile([BS, 1], f32)
            nc.vector.tensor_scalar(out=nxws[:, :], in0=xws[:, :], scalar1=-1.0,
                                    scalar2=None, op0=ALU.mult)
            qsq = sb.tile([BS, C], f32)     # (ur - xws)^2 on ACT, parallel with DVE
            nc.scalar.activation(out=qsq[:, :], in_=dis_ps[:, 0:C], func=ACT.Square,
                                 bias=nxws[:, 0:1])
            gq2 = sb.tile([BS, C], f32)     # gam*qsq - s2
            nc.vector.tensor_scalar(out=gq2[:, :], in0=qsq[:, :], scalar1=gam[:, 0:1],
                                    scalar2=s2[:, 0:1], op0=ALU.mult, op1=ALU.subtract)
            m = sb.tile([BS, C], f32)
            nc.vector.tensor_tensor(out=m[:, :], in0=dis_ps[:, C:2 * C],
                                    in1=gq2[:, :], op=ALU.subtract)

            # rowsum via ACT accum; true-class element via mask+reduce (parallel)
            dis = sb.tile([BS, C], f32)
            rowsum = sb.tile([BS, 1], f32)
            nc.scalar.activation(out=dis[:, :], in_=m[:, :], func=ACT.Sqrt,
                                 accum_out=rowsum[:, :])
            mh = sb.tile([BS, C], f32)
            nc.vector.tensor_tensor(out=mh[:, :], in0=m[:, :], in1=oh[:, :], op=ALU.mult)
            mt = sb.tile([BS, 1], f32)
            nc.vector.tensor_reduce(out=mt[:, :], in_=mh[:, :], axis=AX.X, op=ALU.add)
            intra = sb.tile([BS, 1], f32)
            nc.scalar.sqrt(out=intra[:, :], in_=mt[:, :])
            rs1 = sb.tile([BS, 1], f32)
            nc.vector.tensor_scalar(out=rs1[:, :], in0=rowsum[:, :], scalar1=-1.0 / (C - 1),
                                    scalar2=None, op0=ALU.mult)
            loss = sb.tile([BS, 1], f32)
            nc.vector.tensor_scalar(out=loss[:, :], in0=intra[:, :],
                                    scalar1=float(C) / (C - 1), scalar2=rs1[:, 0:1],
                                    op0=ALU.mult, op1=ALU.add)
            nc.sync.dma_start(loss_d[:, 0:1], loss[:, :])

    nc.finalize()
    return nc


def _get_nc():
    if "nc" not in _cache:
        _cache["nc"] = _build()
    return _cache["nc"]


def _host_precompute(feature_center):
    fc = np.asarray(feature_center, dtype=np.float64)
    g = fc.reshape(C, POOL, G).mean(axis=2)                  # [100, 8]
    yn = g / (np.linalg.norm(g, axis=1, keepdims=True) + 1e-6)
    ybar = yn.mean(axis=0)
    z = yn - ybar
    A = (z.T @ z) / (2 * C - 1)
    M = np.linalg.inv(A)
    M = 0.5 * (M + M.T)
    r = yn @ M                                               # [100, 8]  M y_c
    c0 = M @ ybar
    k2 = np.einsum('cd,ce,de->c', z, z, M)                   # z_c M z_c

    cp = np.zeros((POOL + 1, NCONST), dtype=np.float64)
    cp[0:POOL, 0] = ybar
    cp[0:POOL, 1:1 + C] = r.T
    cp[0:POOL, 1 + C:1 + 2 * C] = -2.0 * r.T
    cp[POOL, 1 + C:1 + 2 * C] = k2
    cp[0:POOL, 1 + 2 * C] = c0
    cp[0:POOL, 1 + 2 * C + 1:1 + 2 * C + 1 + POOL] = M
    return cp.astype(np.float32)


def kernel(hidden, feature_center, y):
    from concourse import bass_utils

    ha = np.empty((B, D + 1), dtype=np.float32)
    ha[:, 0:D] = np.asarray(hidden, dtype=np.float32)
    ha[:, D] = np.asarray(y).astype(np.float32)
    cp = _host_precompute(feature_center)

    nc = _get_nc()
    in_maps = []
    for c in range(NCORES):
        in_maps.append({
            "hidden_in": ha[c * BS:(c + 1) * BS],
            "const_in": cp,
        })
    res = bass_utils.run_bass_kernel_spmd(nc, in_maps, core_ids=list(range(NCORES)))
    loss = np.concatenate([r["loss_out"][:, 0] for r in res.results])
    return np.float32(loss.mean())


# revision 16
# speedup vs baseline: 1.2488x; 1.0044x over previous
"""Trainium2 Bass kernel for nn_ContrastiveCenterLoss_M.

Math reduction
--------------
reference computes, per sample b and class c, a Mahalanobis distance between
the pooled-normalized hidden vector x_b (8-dim) and pooled-normalized class
center y_c (8-dim), where the 8x8 covariance is over the 200 points
{x_b (repeated 100x), y_0..y_99}:

    cov_b = A + beta d_b d_b^T,  A = S_y/199,  d_b = x_b - ybar,  beta = 50/199

A depends only on feature_center and is well-conditioned (cond ~1.9), so
pinv == inv and Sherman-Morrison collapses the per-sample pinv to a rank-1
correction of the shared M = inv(A).  Working in u = x - ybar coordinates
(all class-only terms folded into host constants):

    ur_c  = u.(M y_c)                     [one 9x128^T @ 9x209 matmul]
    e0    = u.(M ybar)                    [extra matmul column]
    uw    = u.M.u = x.w - e0,  w = M u    [w = 8 extra matmul columns]
    gamma = 1/(1/beta + uw),   sg = sqrt(gamma)
    m[b,c] = (uw + 2 e0) + (k2_c - 2 ur_c) - (sg*ur_c - sg*(uw+e0))^2
    k2_c  = (y_c-ybar).M.(y_c-ybar)       [host]
    dis = sqrt(m);  loss_b = (C*dis[b,y_b] - sum_c dis[b,c])/(C-1)

Host precomputes the tiny center-only constants in float64; the device does
all per-sample work.  Data-parallel over batch: 8 cores x 128 samples.
ACT-table sqrt measured at ~1e-6 rel on HW, so no Newton refinement.
NOTE: InstTensorTensorReduce and [p,1]-shaped DRAM outputs crash the exec
unit on this runtime -- avoided (tt+reduce pairs; [128,100] output).
"""

import sys

if "/opt/trn_rl_repo" not in sys.path:
    sys.path.insert(0, "/opt/trn_rl_repo")

import numpy as np

B = 1024
D = 512
C = 100
POOL = 8
G = D // POOL          # 64
NCORES = 8
BS = B // NCORES       # 128 samples per core
BETA = (C / 2) / (2 * C - 1)   # 50/199
NCONST = 1 + 209       # [ybar9 | rhsU(9x209)] packed columns

_cache = {}


def _build():
    import concourse.mybir as mybir
    import concourse.tile as tile
    from concourse import bacc
    from concourse.masks import make_identity

    f32 = mybir.dt.float32
    ALU = mybir.AluOpType
    ACT = mybir.ActivationFunctionType
    AX = mybir.AxisListType
    HALF = D // 2

    nc = bacc.Bacc(
        "TRN2",
        target_bir_lowering=False,
        debug=False,
        enable_asserts=False,
        num_devices=NCORES,
    )

    hidden_d = nc.dram_tensor("hidden_in", [BS, D + 1], f32, kind="ExternalInput")
    const_d = nc.dram_tensor("const_in", [POOL + 1, NCONST], f32, kind="ExternalInput")
    loss_d = nc.dram_tensor("loss_out", [BS, C], f32, kind="ExternalOutput")

    with tile.TileContext(nc) as tc:
        with (
            tc.tile_pool(name="sb", bufs=1) as sb,
            tc.tile_pool(name="ps", bufs=1, space="PSUM") as ps,
        ):
            # ACT-table ordering hint: make the first ACT op a Sqrt so walrus
            # loads the sqrt set (which also contains square) exactly once,
            # early, overlapped with the DMA.
            warm = sb.tile([1, 1], f32)
            nc.vector.memset(warm[:, :], 1.0)
            nc.scalar.sqrt(out=warm[:, :], in_=warm[:, :])

            # hidden + labels (y packed as f32 col 512), asymmetric split: the
            # second (later-arriving) DMA is kept small so pooling finishes sooner
            SP1 = 6 * G
            h1 = sb.tile([BS, SP1], f32)
            h2 = sb.tile([BS, D - SP1 + 1], f32)
            nc.sync.dma_start(h1[:, :], hidden_d[:, 0:SP1])
            nc.sync.dma_start(h2[:, :], hidden_d[:, SP1:D + 1])
            ylab = h2[:, D - SP1:D - SP1 + 1]
            cst = sb.tile([POOL + 1, NCONST], f32)
            nc.sync.dma_start(cst[:, :], const_d[:, :])
            nybar9 = cst[:, 0:1]   # holds -ybar
            rhsU = cst[:, 1:1 + 209]

            # constants with no deps: identity (PE transpose) + iota (one-hot)
            ident = sb.tile([BS, BS], f32)
            make_identity(nc, ident[:, :])
            io_f = sb.tile([BS, C], f32)
            nc.gpsimd.iota(out=io_f[:, :], pattern=[[1, C]], base=0,
                           channel_multiplier=0, allow_small_or_imprecise_dtypes=True)

            # ---- pool hidden into 8 groups of 64, L2-normalize -> x ----
            s8 = sb.tile([BS, POOL], f32)
            nc.vector.tensor_reduce(
                out=s8[:, 0:6],
                in_=h1[:, :].rearrange("p (g e) -> p g e", e=G),
                axis=AX.X, op=ALU.add,
            )
            nc.vector.tensor_reduce(
                out=s8[:, 6:POOL],
                in_=h2[:, 0:D - SP1].rearrange("p (g e) -> p g e", e=G),
                axis=AX.X, op=ALU.add,
            )
            sq = sb.tile([BS, POOL], f32)
            ss = sb.tile([BS, 1], f32)
            nc.scalar.activation(
                out=sq[:, :], in_=s8[:, :], func=ACT.Square, scale=1.0 / G,
                accum_out=ss[:, :],
            )
            nv = sb.tile([BS, 1], f32)
            nc.scalar.sqrt(out=nv[:, :], in_=ss[:, :])
            ne = sb.tile([BS, 1], f32)
            nc.vector.tensor_scalar(out=ne[:, :], in0=nv[:, :], scalar1=1e-6,
                                    scalar2=None, op0=ALU.add)
            rn = sb.tile([BS, 1], f32)
            nc.vector.reciprocal(out=rn[:, :], in_=ne[:, :])
            xn9 = sb.tile([BS, POOL + 1], f32)   # [x | 1]
            nc.vector.tensor_scalar(
                out=xn9[:, 0:POOL], in0=s8[:, :], scalar1=1.0 / G, scalar2=rn[:, 0:1],
                op0=ALU.mult, op1=ALU.mult,
            )
            nc.vector.memset(xn9[:, POOL:POOL + 1], 1.0)

            # one-hot mask of the true class: off the critical path
            oh = sb.tile([BS, C], f32)
            nc.vector.tensor_scalar(out=oh[:, :], in0=io_f[:, :], scalar1=ylab[:, 0:1],
                                    scalar2=None, op0=ALU.is_equal)

            # ---- u^T (+ones row) = transpose(x|1) - (ybar|0) ----
            xnt_ps = ps.tile([POOL + 1, BS], f32)
            nc.tensor.transpose(xnt_ps[:, :], xn9[:, :], ident[:, :])
            ut9 = sb.tile([POOL + 1, BS], f32)
            nc.scalar.activation(out=ut9[:, :], in_=xnt_ps[:, :], func=ACT.Identity,
                                 bias=nybar9)

            # ---- the contraction, split so the small w/e0 block lands first
            # and the per-sample scalar chain overlaps the big 200-col matmul.
            # wps col 0 = e0 = u.(M ybar) ; cols 1:9 = w = M u
            # dis_ps cols 0:100 = u.r_c ; 100:200 = k2_c - 2 u.r_c
            wps = ps.tile([BS, 9], f32)
            nc.tensor.matmul(wps[:, :], ut9[:, :], rhsU[:, 200:209])
            dis_ps = ps.tile([BS, 200], f32)
            nc.tensor.matmul(dis_ps[:, :], ut9[:, :], rhsU[:, 0:200])

            # ---- per-sample scalars (note uw + e0 = u.M.x = x.w = xws) ----
            xw = sb.tile([BS, POOL], f32)
            nc.vector.tensor_tensor(out=xw[:, :], in0=xn9[:, 0:POOL],
                                    in1=wps[:, 1:9], op=ALU.mult)
            xws = sb.tile([BS, 1], f32)
            nc.vector.tensor_reduce(out=xws[:, :], in_=xw[:, :], axis=AX.X, op=ALU.add)
            e0 = sb.tile([BS, 1], f32)
            nc.vector.tensor_copy(out=e0[:, :], in_=wps[:, 0:1])
            den = sb.tile([BS, 1], f32)    # 1/beta + uw = (xws - e0) + 1/beta
            nc.vector.tensor_scalar(out=den[:, :], in0=xws[:, :], scalar1=e0[:, 0:1],
                                    scalar2=1.0 / BETA, op0=ALU.subtract, op1=ALU.add)
            gam = sb.tile([BS, 1], f32)
            nc.vector.reciprocal(out=gam[:, :], in_=den[:, :])
            s2 = sb.tile([BS, 1], f32)     # uw + 2 e0 = xws + e0
            nc.vector.tensor_scalar(out=s2[:, :], in0=xws[:, :], scalar1=e0[:, 0:1],
                                    scalar2=None, op0=ALU.add)

            # ---- m = (k2 - 2ur) - (gam*(ur - xws)^2 - s2) ; dis = sqrt(m) ----
            nxws = sb.tile([BS, 1], f32)
            nc.vector.tensor_scalar(out=nxws[:, :], in0=xws[:, :], scalar1=-1.0,
                                    scalar2=None, op0=ALU.mult)
            qsq = sb.tile([BS, C], f32)     # (ur - xws)^2 on ACT, parallel with DVE
            nc.scalar.activation(out=qsq[:, :], in_=dis_ps[:, 0:C], func=ACT.Square,
                                 bias=nxws[:, 0:1])
            gq2 = sb.tile([BS, C], f32)     # gam*qsq - s2
            nc.vector.tensor_scalar(out=gq2[:, :], in0=qsq[:, :], scalar1=gam[:, 0:1],
                                    scalar2=s2[:, 0:1], op0=ALU.mult, op1=ALU.subtract)
            m = sb.tile([BS, C], f32)
            nc.vector.tensor_tensor(out=m[:, :], in0=dis_ps[:, C:2 * C],
                                    in1=gq2[:, :], op=ALU.subtract)

            # rowsum via ACT accum; true-class element via mask+reduce (parallel)
            dis = sb.tile([BS, C], f32)
            rowsum = sb.tile([BS, 1], f32)
            nc.scalar.activation(out=dis[:, :], in_=m[:, :], func=ACT.Sqrt,
                                 accum_out=rowsum[:, :])
            mh = sb.tile([BS, C], f32)
            nc.vector.tensor_tensor(out=mh[:, :], in0=m[:, :], in1=oh[:, :], op=ALU.mult)
            mt = sb.tile([BS, 1], f32)
            nc.vector.tensor_reduce(out=mt[:, :], in_=mh[:, :], axis=AX.X, op=ALU.add)
            intra = sb.tile([BS, 1], f32)
            nc.scalar.sqrt(out=intra[:, :], in_=mt[:, :])
            rs1 = sb.tile([BS, 1], f32)
            nc.vector.tensor_scalar(out=rs1[:, :], in0=rowsum[:, :], scalar1=-1.0 / (C - 1),
                                    scalar2=None, op0=ALU.mult)
            loss = sb.tile([BS, 1], f32)
            nc.vector.tensor_scalar(out=loss[:, :], in0=intra[:, :],
                                    scalar1=float(C) / (C - 1), scalar2=rs1[:, 0:1],
                                    op0=ALU.mult, op1=ALU.add)
            nc.sync.dma_start(loss_d[:, 0:1], loss[:, :])

    nc.finalize()
    return nc


def _get_nc():
    if "nc" not in _cache:
        _cache["nc"] = _build()
    return _cache["nc"]


def _host_precompute(feature_center):
    fc = np.asarray(feature_center, dtype=np.float64)
    g = fc.reshape(C, POOL, G).mean(axis=2)                  # [100, 8]
    yn = g / (np.linalg.norm(g, axis=1, keepdims=True) + 1e-6)
    ybar = yn.mean(axis=0)
    z = yn - ybar
    A = (z.T @ z) / (2 * C - 1)
    M = np.linalg.inv(A)
    M = 0.5 * (M + M.T)
    r = yn @ M                                               # [100, 8]  M y_c
    c0 = M @ ybar
    k2 = np.einsum('cd,ce,de->c', z, z, M)                   # z_c M z_c

    cp = np.zeros((POOL + 1, NCONST), dtype=np.float64)
    cp[0:POOL, 0] = -ybar
    cp[0:POOL, 1:1 + C] = r.T
    cp[0:POOL, 1 + C:1 + 2 * C] = -2.0 * r.T
    cp[POOL, 1 + C:1 + 2 * C] = k2
    cp[0:POOL, 1 + 2 * C] = c0
    cp[0:POOL, 1 + 2 * C + 1:1 + 2 * C + 1 + POOL] = M
    return cp.astype(np.float32)


def kernel(hidden, feature_center, y):
    from concourse import bass_utils

    ha = np.empty((B, D + 1), dtype=np.float32)
    ha[:, 0:D] = np.asarray(hidden, dtype=np.float32)
    ha[:, D] = np.asarray(y).astype(np.float32)
    cp = _host_precompute(feature_center)

    nc = _get_nc()
    in_maps = []
    for c in range(NCORES):
        in_maps.append({
            "hidden_in": ha[c * BS:(c + 1) * BS],
            "const_in": cp,
        })
    res = bass_utils.run_bass_kernel_spmd(nc, in_maps, core_ids=list(range(NCORES)))
    loss = np.concatenate([r["loss_out"][:, 0] for r in res.results])
    return np.float32(loss.mean())
